# revision 19
# baseline (speedup 1.0000x reference)
"""Trainium2 Bass kernel for masked multi-adaptor LoRA:

    y = x @ W^T + b + sum_n mask[n] * SCALE * ((x @ A[n]^T) @ Bw[n]^T)

Strategy (8 NeuronCores, data-parallel over tokens):
  - Flatten x to [B*S, D] = [16384, 2048] tokens; each core takes T=2048 tokens.
  - x is staged TOKEN-MAJOR: one [D, 128] block per 128-token tile, so tile t
    only needs its own 0.5 MiB x-block plus the streamed W k-tiles. This lets
    the pipeline start ~3us into the kernel instead of waiting for all of x.
  - Device per core, per token tile t:
      h_t[(n,r), tok] = sum_k aT_k.T @ x_t,k      (PE, 64x128, tiny)
      gT_t = h_t * (mask*SCALE)                   (DVE, bf16)
      y_t[tok, o]  = sum_k x_t,k.T @ w_k          (PE, 17th step = rank-64
                                                   LoRA tail: gT_t.T @ bwT)
    Tiles 0,1 run k-outer following the w DMA stream (PSUM holds 2 tiles);
    tiles 2..15 run t-major from resident SBUF; h for tiles (t, t+1) is
    batched just before even tile t, sharing aT LDWEIGHTS.
  - Output is written bf16 (halves drain traffic; adds ~2e-3 rel err) and
    upcast to f32 on host. b is added on host (zeros here).
"""

import os
import sys

if "/opt/trn_rl_repo" not in sys.path:
    sys.path.insert(0, "/opt/trn_rl_repo")

import numpy as np
import ml_dtypes

import concourse.mybir as mybir
import concourse.tile as tile
from concourse import bacc
from concourse.bass_utils import run_bass_kernel_spmd

N_CORES = 8
D = 2048          # d_in
O = 2048          # d_out
T = 2048          # tokens per core (16384 / 8)
NR = 64           # n_adaptors * r = 4 * 16
KT = D // 128     # 16 k-tiles
SCALE = 2.0       # lora_alpha / r = 32 / 16
FREE = 512        # moving-operand width (one matmul output <= one PSUM bank)
NOF = O // FREE   # output column chunks per token tile
NTS = T // 128    # 128-token tiles per core

BF16 = mybir.dt.bfloat16
F32 = mybir.dt.float32

_NC = None


def _build():
    nc = bacc.Bacc("TRN2", target_bir_lowering=False, debug=False)
    # x blocks: xB[p, t*KT*128 + k*128 + tok] = x[t*128+tok, k*128+p]
    xB = nc.dram_tensor("xB", [128, NTS * KT * 128], BF16, kind="ExternalInput").ap()
    wT = nc.dram_tensor("wT", [D, O], BF16, kind="ExternalInput").ap()
    aT = nc.dram_tensor("aT", [128, KT * NR], BF16, kind="ExternalInput").ap()
    bw = nc.dram_tensor("bw", [NR, O], BF16, kind="ExternalInput").ap()
    m64 = nc.dram_tensor("m64", [NR, T], F32, kind="ExternalInput").ap()
    y = nc.dram_tensor("y", [T, O], BF16, kind="ExternalOutput").ap()

    with tile.TileContext(nc) as tc:
        with (
            tc.tile_pool(name="big", bufs=1) as big,
            tc.tile_pool(name="outp", bufs=3) as outp,
            tc.tile_pool(name="psum", bufs=8, space="PSUM") as psum,
        ):
            # ---- SBUF residents; dma issue order = arrival order ----
            # x: tiles 0,1 land individually, k-major per tile; tiles 2..15
            # land in pairs with layout [p, (k, half, tok)] so the h-phase
            # can contract one FD=256 matmul per (pair, k).
            TW = KT * 128
            w_sb = [None] * KT
            xp_sb = [None] * (NTS // 2)
            x_src1 = xB.rearrange("p (t c) -> t p c", t=NTS)
            x_src2 = xB.rearrange("p (q c) -> q p c", q=NTS // 2)
            w_src = wT.rearrange("(k p) o -> k p o", p=128)

            def dma_w(k):
                w_sb[k] = big.tile([128, O], BF16, tag=f"wT{k}", name=f"wT{k}")
                nc.sync.dma_start(w_sb[k], w_src[k])

            x0 = big.tile([128, TW], BF16, tag="xb0")
            nc.sync.dma_start(x0, x_src1[0])
            aT_sb = big.tile([128, KT * NR], BF16, tag="aT_sb")
            nc.sync.dma_start(aT_sb, aT)
            dma_w(0)
            dma_w(1)
            x1 = big.tile([128, TW], BF16, tag="xb1")
            nc.sync.dma_start(x1, x_src1[1])
            m64_sb = big.tile([NR, T], F32, tag="m64_sb")
            nc.sync.dma_start(m64_sb, m64)
            dma_w(2)
            dma_w(3)
            bw_sb = big.tile([NR, O], BF16, tag="bw_sb")
            nc.sync.dma_start(bw_sb, bw)
            for k in range(4, KT):
                dma_w(k)
            for q in range(1, NTS // 2):
                xp = big.tile([128, 2 * TW], BF16, tag=f"xp{q}", name=f"xp{q}")
                nc.sync.dma_start(xp, x_src2[q])
                xp_sb[q] = xp

            gT_sb = big.tile([NR, T], BF16, tag="gT_sb")

            def xk(t, k):
                if t == 0:
                    return x0[:, k * 128:(k + 1) * 128]
                if t == 1:
                    return x1[:, k * 128:(k + 1) * 128]
                base = k * 256 + (t % 2) * 128
                return xp_sb[t // 2][:, base:base + 128]

            # ---- h batches: h = A @ x, gT = h * (mask*SCALE) ----
            def h_single(t, hps):
                # tiles 0,1: separate k-major x blocks, FD=128 per matmul
                for k in range(KT):
                    nc.tensor.matmul(
                        hps, aT_sb[:, k * NR:(k + 1) * NR], xk(t, k),
                        start=(k == 0), stop=(k == KT - 1),
                    )
                sl = slice(t * 128, (t + 1) * 128)
                nc.vector.tensor_mul(gT_sb[:, sl], hps, m64_sb[:, sl])

            def h_pair(q):
                # tiles 2q, 2q+1 interleaved: one FD=256 matmul per k
                hps = psum.tile([NR, 256], F32, tag="ps", name=f"hq{q}")
                for k in range(KT):
                    nc.tensor.matmul(
                        hps, aT_sb[:, k * NR:(k + 1) * NR],
                        xp_sb[q][:, k * 256:(k + 1) * 256],
                        start=(k == 0), stop=(k == KT - 1),
                    )
                sl = slice(q * 256, (q + 1) * 256)
                nc.vector.tensor_mul(gT_sb[:, sl], hps, m64_sb[:, sl])

            def tail(t, ys):
                g_sl = gT_sb[:, t * 128:(t + 1) * 128]
                for o in range(NOF):
                    nc.tensor.matmul(
                        ys[o], g_sl, bw_sb[:, o * FREE:(o + 1) * FREE],
                        start=False, stop=True,
                    )

            def drain(t, ys):
                # one output DMA per tile (keeps queue/sem count low); the
                # last tile drains per-chunk so the final DMA is small.
                split = t == NTS - 1
                ot = outp.tile([128, O], BF16, tag="out", name=f"o{t}")
                for o in range(NOF):
                    sl = slice(o * FREE, (o + 1) * FREE)
                    eng = nc.vector.tensor_copy if o % 2 == 0 else nc.scalar.copy
                    eng(ot[:, sl], ys[o])
                    if split:
                        nc.sync.dma_start(
                            y[t * 128:(t + 1) * 128, sl], ot[:, sl]
                        )
                if not split:
                    nc.sync.dma_start(y[t * 128:(t + 1) * 128, :], ot)

            # ---- tiles 0,1: k-outer, following the w DMA stream; h rides
            # along so the PE has work while the first w k-tiles land.
            # (h psum tiles must be pool-allocated before the 8 y tiles.)
            h0_ps = psum.tile([NR, 128], F32, tag="ps", name="h0")
            h1_ps = psum.tile([NR, 128], F32, tag="ps", name="h1")
            first = [
                [
                    psum.tile([128, FREE], F32, tag="ps", name=f"y{t}_{o}")
                    for o in range(NOF)
                ]
                for t in range(2)
            ]

            def kstep(t, k):
                lhsT = xk(t, k)
                for o in range(NOF):
                    nc.tensor.matmul(
                        first[t][o], lhsT,
                        w_sb[k][:, o * FREE:(o + 1) * FREE],
                        start=(k == 0), stop=False,
                    )

            h_single(0, h0_ps)
            kstep(0, 0)
            kstep(0, 1)
            h_single(1, h1_ps)
            kstep(1, 0)
            kstep(1, 1)
            for k in range(2, KT):
                kstep(0, k)
                kstep(1, k)
            for t in range(2):
                tail(t, first[t])
            for t in range(2):
                drain(t, first[t])

            # ---- tiles 2..15: t-major from resident SBUF ----
            for t in range(2, NTS):
                if t % 2 == 0:
                    h_pair(t // 2)
                ys = [
                    psum.tile([128, FREE], F32, tag="ps", name=f"y{t}_{o}")
                    for o in range(NOF)
                ]
                for k in range(KT):
                    lhsT = xk(t, k)
                    for o in range(NOF):
                        nc.tensor.matmul(
                            ys[o], lhsT, w_sb[k][:, o * FREE:(o + 1) * FREE],
                            start=(k == 0), stop=False,
                        )
                if t < NTS - 1:
                    tail(t, ys)
                    drain(t, ys)
                else:
                    # last tile: interleave tail/cast/dma per chunk so the
                    # final output DMA starts as early as possible.
                    g_sl = gT_sb[:, t * 128:(t + 1) * 128]
                    ot = outp.tile([128, O], BF16, tag="out", name=f"o{t}")
                    for o in range(NOF):
                        sl = slice(o * FREE, (o + 1) * FREE)
                        nc.tensor.matmul(
                            ys[o], g_sl, bw_sb[:, sl], start=False, stop=True
                        )
                        eng = (
                            nc.vector.tensor_copy if o % 2 == 0
                            else nc.scalar.copy
                        )
                        eng(ot[:, sl], ys[o])
                        nc.sync.dma_start(y[t * 128:(t + 1) * 128, sl], ot[:, sl])

    nc.compile()
    return nc


def _get_nc():
    global _NC
    if _NC is None:
        _NC = _build()
    return _NC


def _install_ntff_shim():
    """Optional: register the axon NTFF profile hook so trace=True works."""
    import types
    import antenv
    if "antenv.axon_hooks" in sys.modules:
        return
    hook = [None]
    mod = types.ModuleType("antenv.axon_hooks")
    mod.set_axon_ntff_profile_hook = lambda h: hook.__setitem__(0, h)
    mod.get_axon_ntff_profile_hook = lambda: hook[0]
    sys.modules["antenv.axon_hooks"] = mod
    antenv.axon_hooks = mod
    from trn_agent_boot.trn_boot import _ntff_profile_via_ctypes
    mod.set_axon_ntff_profile_hook(
        _ntff_profile_via_ctypes("/opt/axon/libaxon_pjrt.so")
    )
    from concourse import bass_utils
    bass_utils.upload_artifacts = lambda tmpdir: tmpdir


def kernel(x, mask, W, b, A, Bw):
    x = np.asarray(x)
    mask = np.asarray(mask)
    W = np.asarray(W)
    b = np.asarray(b)
    A = np.asarray(A)
    Bw = np.asarray(Bw)

    B_, S, _ = x.shape
    bf16 = ml_dtypes.bfloat16

    xt = x.reshape(B_ * S, D).astype(bf16)               # [16384, D]
    WT = np.ascontiguousarray(W.astype(bf16).T)          # [D, O]
    # packed A: aT[p, k*64+r] = A_cat[r, k*128+p]
    AT = np.ascontiguousarray(
        A.reshape(NR, KT, 128).transpose(2, 1, 0).reshape(128, KT * NR)
    ).astype(bf16)
    BWT = np.ascontiguousarray(
        Bw.transpose(0, 2, 1).reshape(NR, O).astype(bf16)
    )                                                    # [NR, O]
    m2 = (mask.reshape(mask.shape[0], -1) * np.float32(SCALE)).astype(np.float32)
    m64_full = np.repeat(m2, NR // mask.shape[0], axis=0)  # [NR, 16384]

    in_maps = []
    for c in range(N_CORES):
        sl = slice(c * T, (c + 1) * T)
        xc = xt[sl]  # [T, D]
        # tiles 0,1: [p, (k, tok)]; tiles 2..15 in pairs: [p, (k, half, tok)]
        singles = (
            xc[:256].reshape(2, 128, KT, 128).transpose(0, 3, 2, 1)
            .reshape(2 * 128, KT * 128)
        )  # [(t p), (k tok)] for t in {0, 1}
        pairs = (
            xc[256:].reshape(7, 2, 128, KT, 128).transpose(0, 4, 3, 1, 2)
            .reshape(7, 128, KT * 2 * 128)
        )  # [q, p, (k half tok)]
        xb = np.empty((128, NTS * KT * 128), dtype=bf16)
        xb[:, 0:2048] = singles[:128]
        xb[:, 2048:4096] = singles[128:]
        xb[:, 4096:] = pairs.transpose(1, 0, 2).reshape(128, 7 * KT * 256)
        in_maps.append({
            "xB": xb,
            "wT": WT,
            "aT": AT,
            "bw": BWT,
            "m64": np.ascontiguousarray(m64_full[:, sl]),
        })

    nc = _get_nc()
    trace = os.environ.get("KERNEL_TRACE") == "1"
    if trace:
        try:
            _install_ntff_shim()
        except Exception as e:  # profiling is best-effort
            print(f"NTFF shim unavailable: {e}", file=sys.stderr)
            trace = False
    res = run_bass_kernel_spmd(
        nc, in_maps, core_ids=list(range(N_CORES)), trace=trace
    )
    kernel.last_exec_time_ns = res.exec_time_ns
    kernel.last_trace = res.instructions_and_trace

    yf = np.concatenate(
        [res.results[c]["y"].astype(np.float32) for c in range(N_CORES)], axis=0
    )
    yf = yf + b.astype(np.float32)[None, :]
    return yf.reshape(B_, S, O)
